# revision 56
# baseline (speedup 1.0000x reference)
"""Trainium2 Bass kernel for nn_DecoderLSTMAttention (gate-sharded LSTM).

Math (the reference softmax is over a singleton axis, so attention weights
are identically 1 and context == features broadcast):

    x        = concat([features[:, None, :], embed[captions[:, :-1]]], 1)   # (B,S,E)
    xg       = x @ W_ih.T + (b_ih + b_hh)                                   # (B,S,4H)
    h_t, c_t = lstm_step(xg_t, h_{t-1}, c_{t-1}; W_hh)                      # gates i,f,g,o
    out      = lstm_out @ out_W.T + features @ out_W.T + out_b              # (B,S,V)

Sharding: MODEL-parallel. Each of the 8 cores owns a 128-dim slice of the
hidden state (gate columns i,f,o,g for hid dims [128c, 128c+128)) and a
4000-wide slice of the vocab.  Per step, each core computes its h slice for
ALL 64 batches (32 recurrence matmuls, N=64), then the full h_t is
reassembled on every core with an 8-rank AllGather (16 KB bf16 per rank).
The vocab GEMM (stationary = gathered h history, moving = out_W^T slice)
fills the PE during each step's AllGather window.  features @ out_W^T is
step-invariant and computed once upfront ("featterm"); it is added in the
PSUM->SBUF output stage.

hid-dim permute: the gathered h is chunked mod-8 (chunk k = hid dims
== k (mod 8), partition p = hid//8) so the AllGather-result -> SBUF DMA is
1 KB-contiguous on both sides.  W_hh^T / out_W^T / features rows are
pre-permuted on the host to match ("(p k) ..." reshape).
"""

import numpy as np
import ml_dtypes

import concourse.bass as bass
import concourse.tile as tile
from concourse import bacc, mybir
from concourse.bass_utils import run_bass_kernel_spmd

BF16 = mybir.dt.bfloat16
FP8 = mybir.dt.float8e4
F32 = mybir.dt.float32
AF = mybir.ActivationFunctionType

# W_ih / W_hh are stored fp8e4m3, pre-scaled by GS so values (std 0.02) land
# in the normal range.  PSUM accumulates GS*gates; the gate activations
# descale via their scale= input multiplier.
GS = 16.0

P = 128
B = 64          # all batches on every core
S = 64
H = 1024
NK = 8          # hid chunks of 128
V = 32000
NC = 8          # cores
VC = V // NC    # vocab slice per core: 4000
NVB = 8         # vocab blocks per core
VB = VC // NVB  # 500
T = S * B       # 4096 tokens, tau = t*B + b
NG = 32         # 2-step token groups for the vocab GEMM
WIN = 16        # h history window (slots)


def gate_cols(c):
    """Column indices (into 4H) of core c's gate slice, order [i, f, o, g]."""
    base = np.arange(P) + c * P
    return np.concatenate([base, H + base, 3 * H + base, 2 * H + base])


def emit_body(tc, io):
    nc = tc.nc

    import contextlib
    ctx = contextlib.ExitStack()
    with ctx:
        state = ctx.enter_context(tc.tile_pool(name="state", bufs=1))
        xt_pool = ctx.enter_context(tc.tile_pool(name="xtp", bufs=3))
        ew_pool = ctx.enter_context(tc.tile_pool(name="ewp", bufs=2))
        stage_pool = ctx.enter_context(tc.tile_pool(name="stg", bufs=6))
        dram = ctx.enter_context(tc.tile_pool(name="dram", bufs=2, space="DRAM"))
        # psB 5 banks: the Tile scheduler orders B-chunk matmuls by
        # compile-time readiness, and a chunk's start-MM is only "ready"
        # once the drain-add of the chunk 3 banks earlier is scheduled --
        # with 3 banks that lands after the next recurrence, pushing all B
        # work into alternating windows.  5 banks decouple the pipeline.
        ps_a = ctx.enter_context(tc.tile_pool(name="psa", bufs=2, space="PSUM"))
        ps_b = ctx.enter_context(tc.tile_pool(name="psb", bufs=5, space="PSUM"))
        gps_pool = ctx.enter_context(tc.tile_pool(name="gps", bufs=1, space="PSUM"))

        # ---- resident tensors ----
        wih_sb = state.tile([P, NK, 512], FP8, tag="wih_sb")
        nc.sync.dma_start(wih_sb[:], io["wih"][:])
        whh_sb = state.tile([P, NK, 512], FP8, tag="whh_sb")
        nc.sync.dma_start(whh_sb[:], io["whh"][:])
        bias_sb = state.tile([P, 4], F32, tag="bias_sb")
        nc.sync.dma_start(bias_sb[:], io["bias"][:])
        feat2_sb = state.tile([P, NK, 2, B], BF16, tag="feat2_sb")
        nc.sync.dma_start(feat2_sb[:], io["feat2"][:])
        ident_sb = state.tile([P, P], FP8, tag="ident_sb")
        nc.sync.dma_start(ident_sb[:], io["ident"][:])

        outw_sb = state.tile([P, NK, VC], BF16, tag="outw_sb")
        xg_sb = state.tile([P, 4, T], BF16, tag="xg_sb")
        # hwin: DMA-contiguous layout feeding the recurrence (critical path).
        # hist: [P, NK, WIN, B] copy for the vocab-GEMM stationary -- its
        # [:, kk, w:w+2, :] slice collapses to one contiguous free dim (the
        # BIR verifier requires stationary APs to be single-free-dim).
        hwin = state.tile([P, WIN, NK, B], BF16, tag="hwin")
        hist = state.tile([P, NK, WIN, B], BF16, tag="hist")
        ftv_sb = state.tile([P, NVB, VB], BF16, tag="ftv_sb")
        c_sb = state.tile([P, B], F32, tag="c_sb")
        nc.any.memset(c_sb[:], 0.0)

        # ---- GEMM A block emitter: xg[:, :, blk*512:...] ----
        xtb_pref = {}

        def prefetch_xt(blk):
            xtb = xt_pool.tile([P, NK, 512], BF16, tag="xtb", name="xtb")
            nc.sync.dma_start(xtb[:], io["xt"][blk])
            xtb_pref[blk] = xtb

        def emit_a_block(blk):
            if blk in xtb_pref:
                xtb = xtb_pref.pop(blk)
            else:
                xtb = xt_pool.tile([P, NK, 512], BF16, tag="xtb", name="xtb")
                nc.sync.dma_start(xtb[:], io["xt"][blk])
            for mp in range(4):
                psA = ps_a.tile([P, 512], F32, tag="psA", name="psA")
                for kk in range(NK):
                    nc.tensor.matmul(
                        psA[:], wih_sb[:, kk, mp * P:(mp + 1) * P],
                        xtb[:, kk, :], start=(kk == 0), stop=(kk == NK - 1))
                nc.scalar.activation(
                    xg_sb[:, mp, blk * 512:(blk + 1) * 512], psA[:],
                    AF.Identity, bias=bias_sb[:, mp:mp + 1])

        prefetch_xt(0)
        prefetch_xt(1)
        # outw loads queue on the sync DMA ring right after the first two
        # xt blocks, so the early A blocks and step chain aren't blocked.
        for kk in range(NK):
            nc.sync.dma_start(outw_sb[:, kk], io["outw"][:, kk])
        emit_a_block(0)

        def emit_inject(t, gps):
            nc.tensor.matmul(gps[:], ident_sb[:],
                             xg_sb[:, :, t * B:(t + 1) * B],
                             start=True, stop=(t == 0))

        def emit_rec(t, gps):
            # Two phases over k so phase 0 starts as soon as the first half
            # of the gathered h lands; per-gate stop groups so the sigmoid
            # (needs i,f,o = ml 0..2) can start under the tail matmuls.
            for ph in range(2):
                for ml in range(4):
                    for kk in range(4 * ph, 4 * ph + 4):
                        nc.tensor.matmul(
                            gps[:, ml, :], whh_sb[:, kk, ml * P:(ml + 1) * P],
                            hwin[:, (t - 1) % WIN, kk, :], start=False,
                            stop=(ph == 1 and kk == NK - 1))

        def emit_ew_and_exchange(t, gps):
            # sigmoid(i,f) first so the c-update chain starts while the o
            # sigmoid still runs; each gate tile has its own PSUM stop.
            sif = ew_pool.tile([P, 2, B], F32, tag="sif", name="sif")
            tg = ew_pool.tile([P, B], F32, tag="tg", name="tg")
            so = ew_pool.tile([P, B], F32, tag="so", name="so")
            nc.scalar.activation(sif[:], gps[:, 0:2, :], AF.Sigmoid, scale=1.0 / GS)
            nc.scalar.activation(tg[:], gps[:, 3, :], AF.Tanh, scale=1.0 / GS)
            nc.scalar.activation(so[:], gps[:, 2, :], AF.Sigmoid, scale=1.0 / GS)
            nc.vector.tensor_mul(c_sb[:], sif[:, 1, :], c_sb[:])
            ig = ew_pool.tile([P, B], F32, tag="ig", name="ig")
            nc.vector.tensor_mul(ig[:], sif[:, 0, :], tg[:])
            nc.vector.tensor_add(c_sb[:], c_sb[:], ig[:])
            tc_t = ew_pool.tile([P, B], F32, tag="tc_t", name="tc_t")
            nc.scalar.activation(tc_t[:], c_sb[:], AF.Tanh)
            hst = ew_pool.tile([P, B], BF16, tag="hst", name="hst")
            nc.vector.tensor_mul(hst[:], so[:], tc_t[:])
            snd = dram.tile([P, B], BF16, tag="snd", name="snd")
            rcv = dram.tile([H, B], BF16, tag="rcv", name="rcv",
                            addr_space="Shared")
            nc.gpsimd.dma_start(snd[:], hst[:])
            nc.gpsimd.collective_compute(
                "AllGather", mybir.AluOpType.bypass,
                replica_groups=[list(range(NC))],
                ins=[snd.opt()], outs=[rcv.opt()])
            rcv_pkb = rcv.rearrange("(p k) b -> p k b", p=P)
            nc.gpsimd.dma_start(hwin[:, t % WIN, 0:4, :], rcv_pkb[:, 0:4, :])
            nc.gpsimd.dma_start(hwin[:, t % WIN, 4:8, :], rcv_pkb[:, 4:8, :])

        # step 0 (no recurrence term, h_{-1} == 0)
        gps0 = gps_pool.tile([P, 4, B], F32, tag="gps", name="gps0")
        emit_inject(0, gps0)
        emit_ew_and_exchange(0, gps0)

        # ---- featterm: ftv[:, vb, :] = (features @ out_W.T) dup'd on m ----
        # stationary feat2 [P, 2, B] (=128 cols, batch duplicated), moving
        # outw -> psF [128, VB]; copies to bf16.  Spread across the early
        # steps' AllGather windows (each chunk stalls only on its outw DMA).
        def emit_featterm(vb):
            psF = ps_b.tile([P, VB], F32, tag="psB", name="psF")
            for kk in range(NK):
                nc.tensor.matmul(psF[:], feat2_sb[:, kk, :, :],
                                 outw_sb[:, kk, vb * VB:(vb + 1) * VB],
                                 start=(kk == 0), stop=(kk == NK - 1))
            nc.scalar.activation(ftv_sb[:, vb, :], psF[:], AF.Identity)

        # ---- vocab GEMM chunk emitter ----
        def emit_b_chunk(g, vb):
            w = (2 * g) % WIN
            psB = ps_b.tile([P, VB], F32, tag="psB", name="psB")
            for kk in range(NK):
                nc.tensor.matmul(
                    psB[:], hist[:, kk, w:w + 2, :],
                    outw_sb[:, kk, vb * VB:(vb + 1) * VB],
                    start=(kk == 0), stop=(kk == NK - 1))
            st = stage_pool.tile([P, VB], BF16, tag="st", name="st")
            nc.vector.tensor_add(st[:], psB[:], ftv_sb[:, vb, :])
            # out is [NVB, T, VB] (vb-major) so this store is one contiguous
            # 128 KB run instead of 128 scattered 1 KB descriptors.
            nc.sync.dma_start(io["out"][vb, g * P:(g + 1) * P, :], st[:])

        # ---- steps 1..63 with interleaved A blocks + B chunks ----
        emitted = 0
        for t in range(1, S):
            a_step = t % 2 == 1 and t <= 13
            if a_step:
                emit_a_block((t + 1) // 2)
            # B chunks at the TOP of the step, before the h-gated recurrence:
            # they only touch hist slots >= 2 steps old, so they are ready
            # the moment the PE frees and fill the AllGather-(t-1) window
            # (emitted after rec, the scheduler parks them behind the next
            # recurrence and the PE idles through every other window).
            ready = max(0, ((t - 3) // 2 + 1) * NVB) if t >= 3 else 0
            pace = (2 if a_step else 4) if t < 56 else 6
            target = min(emitted + pace, ready, NG * NVB)
            while emitted < target:
                # priority 0: let the scheduler place these at the earliest
                # dependency-allowed point (their hist slots are >=2 steps
                # old) instead of statically parking them behind the next
                # h-gated recurrence, which leaves the PE idle through
                # every other AllGather window.
                with tc.high_priority():
                    emit_b_chunk(emitted // NVB, emitted % NVB)
                emitted += 1
            # hist copy for slot t-1 on the DVE queue: hwin slot t-1 already
            # landed (rec-t consumes it), so the copy runs during the
            # recurrence matmuls.  (A scattered DMA here would cost ~15us
            # of 128B descriptors.)
            nc.vector.tensor_copy(hist[:, :, (t - 1) % WIN, :],
                                  hwin[:, (t - 1) % WIN, :, :])
            gps = gps_pool.tile([P, 4, B], F32, tag="gps", name="gps")
            emit_inject(t, gps)
            emit_rec(t, gps)
            emit_ew_and_exchange(t, gps)
            if t <= 2:
                # four featterm chunks per early-step window (all eight must
                # be emitted before the first B chunks that read them)
                for vb in range(4 * (t - 1), 4 * t):
                    emit_featterm(vb)

        nc.vector.tensor_copy(hist[:, :, (S - 1) % WIN, :],
                              hwin[:, (S - 1) % WIN, :, :])
        while emitted < NG * NVB:
            with tc.high_priority():
                emit_b_chunk(emitted // NVB, emitted % NVB)
            emitted += 1


# ------------------------------------------------------------------ host ----


def host_prep(features, captions, embed_table, W_ih, W_hh, b_ih, b_hh, out_W):
    bf = ml_dtypes.bfloat16
    f8 = ml_dtypes.float8_e4m3
    features = np.asarray(features, np.float32)
    W_ihT = np.asarray(W_ih, np.float32).T        # (E, 4H)
    W_hhT = np.asarray(W_hh, np.float32).T        # (H, 4H)
    outWT = np.asarray(out_W, np.float32).T       # (H, V)
    b = (np.asarray(b_ih, np.float32) + np.asarray(b_hh, np.float32))

    cap = np.asarray(captions).astype(np.int64)
    x = np.concatenate(
        [features[:, None, :], np.asarray(embed_table, np.float32)[cap[:, :S - 1]]],
        axis=1)                                    # (B, S, E)
    # xt[blk, p, kk, j] = x[b, t, kk*128+p],  tau = t*B + b = blk*512 + j
    # (block-major so each device-side block load is one contiguous run
    # per partition instead of 1024 descriptor-bound 1KB pieces)
    xT = x.transpose(2, 1, 0).reshape(H, T)
    xt = np.ascontiguousarray(
        xT.reshape(NK, P, 8, 512).transpose(2, 1, 0, 3)).astype(bf)
    # feat2[p, kk, j, b] = features[b, 8p+kk]  (mod-8 rows, duplicated j=0,1)
    fT = np.ascontiguousarray(features.T.reshape(P, NK, B))
    feat2 = np.ascontiguousarray(
        np.broadcast_to(fT[:, :, None, :], (P, NK, 2, B))).astype(bf)
    ident = np.eye(P, dtype=f8)

    shards = []
    for c in range(NC):
        cols = gate_cols(c)
        wih = np.ascontiguousarray(
            (W_ihT[:, cols] * GS).reshape(NK, P, 512).transpose(1, 0, 2)
        ).astype(f8)                               # [p, kk, m], e = kk*128+p
        whh = np.ascontiguousarray(
            (W_hhT[:, cols] * GS).reshape(P, NK, 512)).astype(f8)  # hid = 8p+k
        outw = np.ascontiguousarray(
            outWT[:, c * VC:(c + 1) * VC].reshape(P, NK, VC)).astype(bf)
        bias = np.ascontiguousarray(b[cols].reshape(4, P).T) * GS  # [128, 4]
        shards.append({"xt": xt, "wih": wih, "whh": whh, "outw": outw,
                       "bias": bias.astype(np.float32), "feat2": feat2,
                       "ident": ident})
    return shards


def build_program():
    nc = bacc.Bacc("TRN2", target_bir_lowering=False, debug=False,
                   enable_asserts=False)
    io = {
        "xt": nc.dram_tensor("xt", [8, P, NK, 512], BF16,
                             kind="ExternalInput").ap(),
        "wih": nc.dram_tensor("wih", [P, NK, 512], FP8, kind="ExternalInput").ap(),
        "whh": nc.dram_tensor("whh", [P, NK, 512], FP8, kind="ExternalInput").ap(),
        "outw": nc.dram_tensor("outw", [P, NK, VC], BF16, kind="ExternalInput").ap(),
        "bias": nc.dram_tensor("bias", [P, 4], F32, kind="ExternalInput").ap(),
        "feat2": nc.dram_tensor("feat2", [P, NK, 2, B], BF16,
                                kind="ExternalInput").ap(),
        "ident": nc.dram_tensor("ident", [P, P], FP8, kind="ExternalInput").ap(),
        "out": nc.dram_tensor("out", [NVB, T, VB], BF16,
                              kind="ExternalOutput").ap(),
    }
    with tile.TileContext(nc) as tc:
        emit_body(tc, io)
    nc.compile()
    return nc


_CACHE = {}


def _get_program():
    if "nc" not in _CACHE:
        _CACHE["nc"] = build_program()
    return _CACHE["nc"]


def kernel(features, captions, embed_table, W_ih, W_hh, b_ih, b_hh,
           attn_W, attn_b, score_W, score_b, out_W, out_b):
    shards = host_prep(features, captions, embed_table, W_ih, W_hh,
                       b_ih, b_hh, out_W)
    nc = _get_program()
    res = run_bass_kernel_spmd(nc, shards, core_ids=list(range(NC)))
    out = np.empty((B, S, V), np.float32)
    for c in range(NC):
        oc = res.results[c]["out"].astype(np.float32)       # [NVB, T, VB]
        # [vb, t*B+b, v] -> [b, t, vb*VB+v]
        oc = oc.reshape(NVB, S, B, VB).transpose(2, 1, 0, 3).reshape(B, S, VC)
        out[:, :, c * VC:(c + 1) * VC] = oc
    out_b = np.asarray(out_b, np.float32)
    if np.any(out_b):
        out += out_b
    return out
